# revision 6
# baseline (speedup 1.0000x reference)
"""Trainium2 Bass kernel for nn_MultiHeadAttention_76295799046818.

Multi-head attention: B=2, S=2048, D=1024, H=16 heads (d_k=64), causal mask,
fp32 reference.  Sharded over 8 NeuronCores as data-parallel over batch (2) x
tensor-parallel over heads (4 heads per core).  Wq/Wk/Wv are column-parallel;
Wo is row-parallel and each core emits its partial output (its 4 heads'
contribution to the full [S, D] output); the host sums the 4 partials per
batch (the unshard step).  This avoids the on-device all-reduce, which costs
~80-120us in this runtime's collective stack - far more than the extra 6MB of
output DMA.

Device pipeline per core (bf16 matmuls, fp32 PSUM accumulation):
  1. Q^T, K^T projections in [d_k, S] layout (per-partition bias add), V in
     [S, d_k] layout augmented with a ones-column (softmax denominators).
  2. Per (head-pair, q-chunk of 512): S^T = K^T.T Q^T block-matmuls.  The two
     heads of a pair sit at SBUF partitions 0-63 / 64-127, so their K=64
     matmuls occupy disjoint PE row-groups and run concurrently.  The causal
     mask is added inside PSUM via identity x mask-tile matmuls; one ACT exp
     (scale=1/sqrt(d_k) folded in) per 2 k-tiles -> bf16.
  3. AV: out^T[d_k+1, q] = [V|1].T @ expT accumulated over k-tiles; row 64 is
     the softmax denominator.  Denominators are gathered per q-chunk into a
     [4, 512] PSUM tile via one-hot matmuls; 1/den = exp(-ln(den)) on ACT
     (same activation table set as the softmax exp).
  4. Tail: PE broadcast of 1/den rows, DVE multiply, a PE "partition shift"
     matmul packs odd heads into partitions 64..127 of C^T, then the partial
     Wo matmul out^T[j, s] = (Wo[:, o_slice].T).T @ C^T for all 1024 j.
"""

import numpy as np
import ml_dtypes

import concourse.bass as bass
import concourse.mybir as mybir
import concourse.tile as tile
from concourse import bacc
from concourse.bass_utils import run_bass_kernel_spmd

BF16 = ml_dtypes.bfloat16

B, S, D, H, DK = 2, 2048, 1024, 16, 64
N_CORES = 8
TP = 4  # head-parallel degree (per batch)
HPC = H // TP  # heads per core = 4
O = HPC * DK  # output channels per core = 256
NEG = -1000000000.0
KT_BLK = 128
QT_BLK = 512
N_KT = S // KT_BLK  # 16
N_QC = S // QT_BLK  # 4
KC = D // 128  # 8 contraction chunks for projections

_CACHE = {}


def _mask_structure(mask):
    """Classify [KT_BLK x QT_BLK] blocks of the S^T mask.

    Returns (keep[ki][qc] in {'full','skip',int}, mixed_tiles [n,128,512] f32,
    ranges[ki][qc] = (qk_lo, m_hi)): an int indexes mixed_tiles (additive NEG
    pattern), qk_lo is the first q column with any kept element (QK/AV skip
    columns below it), m_hi is one past the last q column with any dropped
    element (the mask matmul covers [0, m_hi)).
    """
    dropped = np.asarray(mask) == 0
    keep, ranges = [], []
    tiles = []
    tile_index = {}
    for ki in range(N_KT):
        row, rrow = [], []
        for qc in range(N_QC):
            sub = dropped[qc * QT_BLK:(qc + 1) * QT_BLK,
                          ki * KT_BLK:(ki + 1) * KT_BLK].T  # [128, 512]
            if not sub.any():
                row.append("full")
                rrow.append((0, 0))
            elif sub.all():
                row.append("skip")
                rrow.append((0, 0))
            else:
                key = sub.tobytes()
                if key not in tile_index:
                    tile_index[key] = len(tiles)
                    tiles.append(np.where(sub, NEG, 0.0).astype(np.float32))
                row.append(tile_index[key])
                col_kept = ~sub.all(axis=0)
                col_drop = sub.any(axis=0)
                qk_lo = int(np.argmax(col_kept))
                m_hi = int(QT_BLK - np.argmax(col_drop[::-1]))
                rrow.append((qk_lo, m_hi))
        keep.append(row)
        ranges.append(rrow)
    if not tiles:
        tiles.append(np.zeros((KT_BLK, QT_BLK), np.float32))
    return keep, np.stack(tiles), ranges


def _build(keep, n_mixed, ranges):
    nc = bacc.Bacc("TRN2", target_bir_lowering=False, debug=False,
                   num_devices=N_CORES)
    dt = mybir.dt
    f32, bf16, f32r = dt.float32, dt.bfloat16, dt.float32r

    def din(name, shape, dtype=bf16):
        return nc.dram_tensor(name, shape, dtype, kind="ExternalInput").ap()

    # all DRAM inputs are pre-packed partition-major on the host so every
    # dma_start is a plain dense copy
    xqt_d = din("xqt", [N_QC, 128, KC, QT_BLK])
    xkt_d = din("xkt", [N_QC, 128, KC, QT_BLK])
    xvt_d = din("xvt", [N_QC, 128, KC, QT_BLK])
    wqt_d = din("wqt", [128, KC, O])
    wkt_d = din("wkt", [128, KC, O])
    wvt_d = din("wvt", [128, KC, O])
    wot_d = din("wot", [128, 2, D])
    bq_d = din("bqc", [128, 2], f32)
    bk_d = din("bkc", [128, 2], f32)
    bvb_d = din("bvb", [128, O], f32)
    bo_d = din("boc", [128, 8], f32)
    um_d = din("umask", [128, n_mixed, QT_BLK])
    oh_d = din("onehot", [16], f32r)  # eye(4) flattened
    bsel_d = din("bsel", [4, HPC, 64], f32r)
    shift_d = din("shiftI", [64, 128])
    id_d = din("ident", [128, 128])
    out_d = nc.dram_tensor("out", [8, 128, S], f32, kind="ExternalOutput").ap()

    EXPF = mybir.ActivationFunctionType.Exp
    LNF = mybir.ActivationFunctionType.Ln

    with tile.TileContext(nc) as tc:
        with (
            tc.tile_pool(name="const", bufs=1) as cpool,
            tc.tile_pool(name="xin", bufs=2) as xpool,
            tc.tile_pool(name="expp", bufs=6) as epool,
            tc.tile_pool(name="small", bufs=3) as spool,
            tc.tile_pool(name="outp", bufs=3) as opool,
        ):
            # hot-path inputs first so compute can start ASAP
            xq0 = xpool.tile([128, KC, QT_BLK], bf16, name="xq", tag="xq")
            nc.sync.dma_start(xq0[:], xqt_d[0])
            xk0 = xpool.tile([128, KC, QT_BLK], bf16, name="xk", tag="xk")
            nc.sync.dma_start(xk0[:], xkt_d[0])
            xv0 = xpool.tile([128, KC, QT_BLK], bf16, name="xv", tag="xv")
            nc.sync.dma_start(xv0[:], xvt_d[0])
            wq_sb = cpool.tile([128, KC, O], bf16, name="wq_sb")
            nc.sync.dma_start(wq_sb[:], wqt_d[:])
            wk_sb = cpool.tile([128, KC, O], bf16, name="wk_sb")
            nc.sync.dma_start(wk_sb[:], wkt_d[:])
            wv_sb = cpool.tile([128, KC, O], bf16, name="wv_sb")
            nc.sync.dma_start(wv_sb[:], wvt_d[:])
            bq_sb = cpool.tile([128, 2], f32, name="bq_sb")
            nc.sync.dma_start(bq_sb[:], bq_d[:])
            bk_sb = cpool.tile([128, 2], f32, name="bk_sb")
            nc.sync.dma_start(bk_sb[:], bk_d[:])
            bvb_sb = cpool.tile([128, O], f32, name="bvb_sb")
            nc.sync.dma_start(bvb_sb[:], bvb_d[:])
            um_sb = cpool.tile([128, n_mixed, QT_BLK], bf16, name="um_sb")
            nc.sync.dma_start(um_sb[:], um_d[:])
            id_sb = cpool.tile([128, 128], bf16, name="id_sb")
            nc.sync.dma_start(id_sb[:], id_d[:])
            oh_sb = cpool.tile([65, 16], f32r, name="oh_sb")
            nc.sync.dma_start(oh_sb[64:65, :], oh_d[:])
            # tail-phase constants
            wo_sb = cpool.tile([128, 2, D], bf16, name="wo_sb")
            nc.sync.dma_start(wo_sb[:], wot_d[:])
            bo_sb = cpool.tile([128, 8], f32, name="bo_sb")
            nc.sync.dma_start(bo_sb[:], bo_d[:])
            shift_sb = cpool.tile([64, 128], bf16, name="shift_sb")
            nc.sync.dma_start(shift_sb[:], shift_d[:])
            bsel_sb = cpool.tile([4, HPC, 64], f32r, name="bsel_sb")
            nc.sync.dma_start(bsel_sb[:], bsel_d[:])

            qt_sb = cpool.tile([128, 2, S], bf16, name="qt_sb")
            kt_sb = cpool.tile([128, 2, S], bf16, name="kt_sb")
            vaug = cpool.tile([128, N_KT, HPC, 66], bf16, name="vaug")
            nc.vector.memset(vaug[:], 1.0)
            ct_sb = cpool.tile([128, 2, S], bf16, name="ct_sb")
            ctraw = cpool.tile([64, HPC * N_QC, QT_BLK], bf16, name="ctraw")
            rden = cpool.tile([4, N_QC, QT_BLK], f32r, name="rden")

            # main phase: projections + attention, interleaved per s-chunk
            # PSUM banks: pp(1) + st(2x2) + pav(2) + pdens(1) = 8
            with tc.tile_pool(name="psA", bufs=2, space="PSUM") as ps:
                for sc in range(N_QC):
                    ssl = bass.ds(sc * QT_BLK, QT_BLK)
                    if sc == 0:
                        xq, xk, xv = xq0, xk0, xv0
                    else:
                        xq = xpool.tile([128, KC, QT_BLK], bf16, name="xq",
                                        tag="xq")
                        nc.sync.dma_start(xq[:], xqt_d[sc])
                        xk = xpool.tile([128, KC, QT_BLK], bf16, name="xk",
                                        tag="xk")
                        nc.sync.dma_start(xk[:], xkt_d[sc])
                        xv = xpool.tile([128, KC, QT_BLK], bf16, name="xv",
                                        tag="xv")
                        nc.sync.dma_start(xv[:], xvt_d[sc])

                    for ot in range(2):
                        osl = bass.ds(ot * 128, 128)
                        pk = ps.tile([128, QT_BLK], f32, name="pk", tag="pp",
                                     bufs=1)
                        for kc in range(KC):
                            nc.tensor.matmul(pk[:], wk_sb[:, kc, osl],
                                             xk[:, kc, :], start=(kc == 0),
                                             stop=(kc == KC - 1))
                        nc.vector.tensor_scalar(kt_sb[:, ot, ssl], pk[:],
                                                bk_sb[:, ot:ot + 1], None,
                                                mybir.AluOpType.add)
                    for mt in range(4):
                        pv = ps.tile([128, O], f32, name="pv", tag="pp",
                                     bufs=1)
                        for kc in range(KC):
                            nc.tensor.matmul(
                                pv[:], xv[:, kc, bass.ds(mt * 128, 128)],
                                wv_sb[:, kc, :], start=(kc == 0),
                                stop=(kc == KC - 1))
                        nc.vector.tensor_tensor(
                            vaug[:, sc * 4 + mt, :, 0:64],
                            pv[:].rearrange("p (h d) -> p h d", h=HPC),
                            bvb_sb[:].rearrange("p (h d) -> p h d", h=HPC),
                            mybir.AluOpType.add)
                    for ot in range(2):
                        osl = bass.ds(ot * 128, 128)
                        pq = ps.tile([128, QT_BLK], f32, name="pq", tag="pp",
                                     bufs=1)
                        for kc in range(KC):
                            nc.tensor.matmul(pq[:], wq_sb[:, kc, osl],
                                             xq[:, kc, :], start=(kc == 0),
                                             stop=(kc == KC - 1))
                        nc.vector.tensor_scalar(qt_sb[:, ot, ssl], pq[:],
                                                bq_sb[:, ot:ot + 1], None,
                                                mybir.AluOpType.add)

                    # attention for q-chunk sc; the two heads of a pair sit
                    # at partitions 0-63 / 64-127 -> their K=64 matmuls use
                    # disjoint PE row-groups and run concurrently
                    qc = sc
                    kis = [ki for ki in range(N_KT) if keep[ki][qc] != "skip"]
                    assert kis, "fully-masked q-chunk unsupported"
                    groups = [kis[i:i + 2] for i in range(0, len(kis), 2)]
                    pdq = ps.tile([4, QT_BLK], f32, name="pdq", tag="pdens",
                                  bufs=1)
                    for hp in range(2):
                        pav = ps.tile([65, 2, QT_BLK], f32, name="pav",
                                      tag="pav", bufs=1)
                        n_av = [0, 0]
                        for g in groups:
                            st2 = [ps.tile([128, 2, QT_BLK], f32, name="st",
                                           tag="st", bufs=2)
                                   for _ in range(2)]
                            et2 = [epool.tile([128, 2, QT_BLK], bf16,
                                              name="et", tag="et")
                                   for _ in range(2)]
                            for gi, ki in enumerate(g):
                                mixed = keep[ki][qc] != "full"
                                qk_lo = 0
                                if mixed:
                                    qk_lo, m_hi = ranges[ki][qc]
                                    for side in range(2):
                                        nc.tensor.matmul(
                                            st2[side][:, gi, 0:m_hi],
                                            id_sb[:],
                                            um_sb[:, keep[ki][qc], 0:m_hi],
                                            start=True, stop=False)
                                for side in range(2):
                                    po = bass.ds(side * 64, 64)
                                    nc.tensor.matmul(
                                        st2[side][:, gi, qk_lo:QT_BLK],
                                        kt_sb[po, hp,
                                              bass.ds(ki * KT_BLK, KT_BLK)],
                                        qt_sb[po, hp,
                                              bass.ds(qc * QT_BLK + qk_lo,
                                                      QT_BLK - qk_lo)],
                                        start=not mixed, stop=True)
                            for side in range(2):
                                if len(g) == 2:
                                    nc.scalar.activation(et2[side][:],
                                                         st2[side][:], EXPF,
                                                         scale=0.125)
                                else:
                                    nc.scalar.activation(et2[side][:, 0, :],
                                                         st2[side][:, 0, :],
                                                         EXPF, scale=0.125)
                            for side in range(2):
                                h = 2 * hp + side
                                for gi, ki in enumerate(g):
                                    av_lo = 0
                                    if (n_av[side] > 0
                                            and keep[ki][qc] != "full"):
                                        av_lo = ranges[ki][qc][0]
                                    nc.tensor.matmul(
                                        pav[:, side, av_lo:QT_BLK],
                                        vaug[:, ki, h, 0:65],
                                        et2[side][:, gi, av_lo:QT_BLK],
                                        start=(n_av[side] == 0),
                                        stop=(n_av[side] == len(kis) - 1),
                                        skip_group_check=True)
                                    n_av[side] += 1
                        for side in range(2):
                            h = 2 * hp + side
                            idx = h * N_QC + qc
                            dsb = spool.tile([65, QT_BLK], f32r, name="dsb",
                                             tag="dsb")
                            nc.vector.tensor_copy(dsb[64:65, :],
                                                  pav[64:65, side, :])
                            nc.tensor.matmul(
                                pdq[:], oh_sb[64:65, bass.ds(h * 4, 4)],
                                dsb[64:65, :],
                                start=(h == 0), stop=(h == HPC - 1),
                                skip_group_check=True)
                            nc.vector.tensor_copy(ctraw[:, idx, :],
                                                  pav[0:64, side, :])
                    # 1/den = exp(-ln(den)) -- same ACT table set as exp
                    dtmp = spool.tile([4, QT_BLK], f32, name="dtmp",
                                      tag="dtmp")
                    nc.scalar.activation(dtmp[:], pdq[:], LNF)
                    with nc.allow_low_precision(reason="f32r softmax denom"):
                        nc.scalar.activation(rden[:, qc, :], dtmp[:], EXPF,
                                             scale=-1.0)

            # tail: normalize + pack C^T + partial Wo, per q-chunk
            # PSUM banks: pnorm(2) + pwo(3)
            with tc.tile_pool(name="psB", bufs=2, space="PSUM") as ps:
                for qc in range(N_QC):
                    qsl = bass.ds(qc * QT_BLK, QT_BLK)
                    for h in range(HPC):
                        hp, ho = h // 2, (h % 2) * 64
                        idx = h * N_QC + qc
                        pbc = ps.tile([64, QT_BLK], f32, name="pbc",
                                      tag="pnorm", bufs=2)
                        nc.tensor.matmul(pbc[:], bsel_sb[:, h, :],
                                         rden[:, qc, :], start=True,
                                         stop=True)
                        if ho == 0:
                            nc.vector.tensor_tensor(ct_sb[0:64, hp, qsl],
                                                    ctraw[:, idx, :], pbc[:],
                                                    mybir.AluOpType.mult)
                        else:
                            scr = spool.tile([64, QT_BLK], bf16, name="scr",
                                             tag="scr")
                            nc.vector.tensor_tensor(scr[:], ctraw[:, idx, :],
                                                    pbc[:],
                                                    mybir.AluOpType.mult)
                            pct = ps.tile([128, QT_BLK], f32, name="pct",
                                          tag="pnorm", bufs=2)
                            nc.tensor.matmul(pct[:], shift_sb[:], scr[:],
                                             start=True, stop=True)
                            nc.vector.tensor_copy(ct_sb[64:128, hp, qsl],
                                                  pct[64:128, :])
                    for jt in range(8):
                        pwo = ps.tile([128, QT_BLK], f32, name="pwo",
                                      tag="pwo", bufs=3)
                        for kc in range(2):
                            nc.tensor.matmul(
                                pwo[:], wo_sb[:, kc, bass.ds(jt * 128, 128)],
                                ct_sb[:, kc, qsl], start=(kc == 0),
                                stop=(kc == 1))
                        osb = opool.tile([128, QT_BLK], f32, name="osb",
                                         tag="osb")
                        if jt % 2 == 0:
                            nc.vector.tensor_scalar(osb[:], pwo[:],
                                                    bo_sb[:, jt:jt + 1],
                                                    None,
                                                    mybir.AluOpType.add)
                        else:
                            nc.scalar.activation(
                                osb[:], pwo[:],
                                mybir.ActivationFunctionType.Identity,
                                bias=bo_sb[:, jt:jt + 1])
                        nc.sync.dma_start(out_d[jt][:, qsl], osb[:])

    nc.compile()
    return nc


def kernel(query, key, value, mask, Wq, bq, Wk, bk, Wv, bv, Wo, bo):
    query = np.asarray(query, np.float32)
    key_ = np.asarray(key, np.float32)
    value = np.asarray(value, np.float32)
    Wq, Wk, Wv, Wo = (np.asarray(w, np.float32) for w in (Wq, Wk, Wv, Wo))
    bq, bk, bv, bo = (np.asarray(b_, np.float32) for b_ in (bq, bk, bv, bo))

    keep, mtiles, ranges = _mask_structure(mask)
    ckey = np.asarray(mask).tobytes()
    if ckey not in _CACHE:
        _CACHE.clear()
        _CACHE[ckey] = _build(keep, len(mtiles), ranges)
    nc = _CACHE[ckey]

    def xt(x):  # [S, D] -> [N_QC, 128, KC, QT_BLK] bf16, partition-major
        a = x.T.reshape(KC, 128, S).transpose(1, 0, 2)  # [128, KC, S]
        a = a.reshape(128, KC, N_QC, QT_BLK).transpose(2, 0, 1, 3)
        return np.ascontiguousarray(a).astype(BF16)

    def wslice(W, c):  # [D, D] -> [128, KC, O] bf16 of W[o_slice].T
        hg = c % TP
        a = W[hg * O:(hg + 1) * O].T.reshape(KC, 128, O).transpose(1, 0, 2)
        return np.ascontiguousarray(a).astype(BF16)

    onehot = np.eye(4, dtype=np.float32).reshape(-1)
    bsel = np.zeros((4, HPC, 64), np.float32)
    for h in range(HPC):
        bsel[h, h, :] = 1.0
    shift = np.zeros((64, 128), np.float32)
    shift[np.arange(64), 64 + np.arange(64)] = 1.0
    ident = np.eye(128, dtype=np.float32)
    um_pm = np.ascontiguousarray(mtiles.transpose(1, 0, 2))  # [128, n, 512]

    in_maps = []
    for c in range(N_CORES):
        b_, hg = c // TP, c % TP
        osl = slice(hg * O, (hg + 1) * O)
        bo_part = bo if hg == 0 else np.zeros_like(bo)
        wot = Wo[:, osl].T.reshape(2, 128, D).transpose(1, 0, 2)
        in_maps.append({
            "xqt": xt(query[b_]),
            "xkt": xt(key_[b_]),
            "xvt": xt(value[b_]),
            "wqt": wslice(Wq, c),
            "wkt": wslice(Wk, c),
            "wvt": wslice(Wv, c),
            "wot": np.ascontiguousarray(wot).astype(BF16),
            "bqc": np.ascontiguousarray(bq[osl].reshape(2, 128).T),
            "bkc": np.ascontiguousarray(bk[osl].reshape(2, 128).T),
            "bvb": np.ascontiguousarray(np.broadcast_to(bv[osl], (128, O))),
            "boc": np.ascontiguousarray(bo_part.reshape(8, 128).T),
            "umask": um_pm.astype(BF16),
            "onehot": onehot,
            "bsel": bsel,
            "shiftI": shift.astype(BF16),
            "ident": ident.astype(BF16),
        })

    res = run_bass_kernel_spmd(nc, in_maps, core_ids=list(range(N_CORES)))

    out = np.zeros((B, S, D), np.float32)
    for c in range(N_CORES):
        part = res.results[c]["out"].reshape(D, S)  # out^T [j, s]
        out[c // TP] += part.T
    return out
